# revision 1
# baseline (speedup 1.0000x reference)
"""Trainium2 Bass kernel for LocalDenseSynthesizerAttention.

Data-parallel over batch B=8 -> 8 cores, one batch each. All projections in
bf16 (PE full rate), fp32 PSUM accumulation. The local window C=45 weighted
sum is computed as banded matmuls: the banded matrix B[s,t'] = attn[t0+t',h,s-t']
is an affine strided view of a zero-padded attn tensor in DRAM, loaded
matmul-ready via XBAR transpose-DMA.

Self-contained: hardcodes shapes from the problem spec.
"""
import sys
sys.path.insert(0, '/opt/trn_rl_repo')
import numpy as np
import ml_dtypes

import concourse.bass as bass
import concourse.mybir as mybir
import concourse.tile as tile
from concourse import bacc
from concourse.bass_utils import run_bass_kernel_spmd

T, F = 2048, 512
H, C, DK = 8, 45, 64
HC = H * C          # 360
W = 128             # padded attn width per head (covers s-t' in [-63,127])
S = 64              # t' band-block size
NB = T // S         # 32 band blocks
PADV = 22           # (C-1)//2
KF = F // 128       # 4 contraction chunks
NT128 = T // 128    # 16

BF16 = mybir.dt.bfloat16
F32 = mybir.dt.float32

_CACHE = {}


def _build():
    nc = bacc.Bacc("TRN2", target_bir_lowering=False, debug=False, num_devices=8)
    qT = nc.dram_tensor("qT", (F, T), BF16, kind="ExternalInput")
    vT = nc.dram_tensor("vT", (F, T), BF16, kind="ExternalInput")
    w1 = nc.dram_tensor("w1", (F, F), BF16, kind="ExternalInput")
    w2 = nc.dram_tensor("w2", (F, HC), BF16, kind="ExternalInput")
    w3 = nc.dram_tensor("w3", (F, F), BF16, kind="ExternalInput")
    wo = nc.dram_tensor("wo", (F, F), BF16, kind="ExternalInput")
    out = nc.dram_tensor("out", (T, F), F32, kind="ExternalOutput")

    with tile.TileContext(nc) as tc:
        with tc.tile_pool(name="wpool", bufs=1) as wp, \
             tc.tile_pool(name="inpool", bufs=1) as inp, \
             tc.tile_pool(name="persist", bufs=1) as pers, \
             tc.tile_pool(name="work", bufs=2) as wk, \
             tc.tile_pool(name="band", bufs=4) as bp, \
             tc.tile_pool(name="psmain", bufs=2, space="PSUM") as psm, \
             tc.tile_pool(name="psband", bufs=4, space="PSUM") as psb, \
             tc.tile_pool(name="drampool", bufs=1, space="DRAM") as dp:

            # ---- weights to SBUF, [128, KF, n] layout (partition = contraction)
            w1_t = wp.tile([128, KF, F], BF16, tag="w1")
            nc.sync.dma_start(w1_t[:], w1[:, :].rearrange("(ko p) n -> p ko n", p=128))
            w2_t = wp.tile([128, KF, HC], BF16, tag="w2")
            nc.sync.dma_start(w2_t[:], w2[:, :].rearrange("(ko p) n -> p ko n", p=128))
            w3_t = wp.tile([128, KF, F], BF16, tag="w3")
            nc.sync.dma_start(w3_t[:], w3[:, :].rearrange("(ko p) n -> p ko n", p=128))
            wo_t = wp.tile([128, KF, F], BF16, tag="wo")
            nc.sync.dma_start(wo_t[:], wo[:, :].rearrange("(ko p) n -> p ko n", p=128))

            # ---- inputs (f-major) to SBUF: 4 folds of [128, T] each
            qT_t = inp.tile([128, KF, T], BF16, tag="qT")
            nc.sync.dma_start(qT_t[:], qT[:, :].rearrange("(ko p) n -> p ko n", p=128))
            vT_t = inp.tile([128, KF, T], BF16, tag="vT")
            nc.sync.dma_start(vT_t[:], vT[:, :].rearrange("(ko p) n -> p ko n", p=128))

            # ---- DRAM scratch
            # vpad rows r = t + PADV; rows [0,22) and [2070,2112) zero
            vpad = dp.tile([T + 64, F], BF16)
            # apad: 1 guard row + 2048 data rows + 1 guard row, row = [8 heads x 128]
            apad = dp.tile([T + 2, H * W], BF16)

            # zero tile for guards / edges
            z_t = pers.tile([128, H * W], BF16, tag="zt")
            nc.any.memzero(z_t[:])
            nc.sync.dma_start(vpad[0:PADV, :], z_t[0:PADV, 0:F])
            nc.sync.dma_start(vpad[T + PADV:T + 64, :], z_t[0:64 - PADV, 0:F])
            nc.sync.dma_start(apad[0:1, :], z_t[0:1, :])
            nc.sync.dma_start(apad[T + 1:T + 2, :], z_t[0:1, :])

            # ---- persistent SBUF activations
            qrT = pers.tile([128, KF, T], BF16, tag="qrT")   # relu(q @ w1), f-major
            xT = pers.tile([128, KF, T], BF16, tag="xT")     # band output, f-major

            # ================= Phase A: q-proj + relu (f-major out) ===========
            for fo in range(KF):
                for tt in range(KF):  # 4 t-tiles of 512
                    ps = psm.tile([128, 512], F32, tag="mm")
                    for k in range(KF):
                        nc.tensor.matmul(
                            ps[:], w1_t[:, k, fo * 128:(fo + 1) * 128],
                            qT_t[:, k, tt * 512:(tt + 1) * 512],
                            start=(k == 0), stop=(k == KF - 1))
                    nc.scalar.activation(qrT[:, fo, tt * 512:(tt + 1) * 512], ps[:],
                                         mybir.ActivationFunctionType.Relu)

            # ================= Phase C: v-proj (t-major out) -> vpad ==========
            for tb in range(NT128):
                ps = psm.tile([128, 512], F32, tag="mm")
                for k in range(KF):
                    nc.tensor.matmul(
                        ps[:], vT_t[:, k, tb * 128:(tb + 1) * 128],
                        w3_t[:, k, :],
                        start=(k == 0), stop=(k == KF - 1))
                v_sb = wk.tile([128, F], BF16, tag="vsb")
                nc.scalar.copy(v_sb[:], ps[:])
                nc.sync.dma_start(vpad[PADV + tb * 128:PADV + (tb + 1) * 128, :], v_sb[:])

            # ====== Phase B: s-proj (t-major) + softmax -> apad (padded) ======
            for tb in range(NT128):
                ps = psm.tile([128, 512], F32, tag="mm")
                for k in range(KF):
                    nc.tensor.matmul(
                        ps[:, 0:HC], qrT[:, k, tb * 128:(tb + 1) * 128],
                        w2_t[:, k, :],
                        start=(k == 0), stop=(k == KF - 1))
                e_t = wk.tile([128, HC], F32, tag="et")
                nc.scalar.activation(e_t[:], ps[:, 0:HC],
                                     mybir.ActivationFunctionType.Exp)
                zs = wk.tile([128, H], F32, tag="zs")
                nc.vector.reduce_sum(zs[:], e_t[:].rearrange("p (h c) -> p h c", c=C),
                                     axis=mybir.AxisListType.X)
                rz = wk.tile([128, H], F32, tag="rz")
                nc.vector.reciprocal(rz[:], zs[:])
                ap_t = wk.tile([128, H * W], BF16, tag="apad")
                if tb < 2:
                    # zero the pad region once per pool slot (bufs=2); the pad
                    # columns are never overwritten afterwards
                    nc.any.memzero(ap_t[:])
                nc.vector.tensor_mul(
                    out=ap_t[:].rearrange("p (h w) -> p h w", w=W)[:, :, 0:C],
                    in0=e_t[:].rearrange("p (h c) -> p h c", c=C),
                    in1=rz[:, :, None].to_broadcast((128, H, C)))
                nc.sync.dma_start(apad[1 + tb * 128:1 + (tb + 1) * 128, :], ap_t[:])

            # ================= Phase D: banded attention matmuls ==============
            # x[t', h*64+d] = sum_s vpad[t0+s, h*64+d] * B_h[s, t']
            # B_h loaded via transpose-DMA of sheared apad view.
            apad_h = apad.tensor  # underlying DRAM handle
            apad_off = apad.offset if isinstance(apad.offset, int) else 0
            for g in range(8):          # groups of 4 band blocks = 256 t'
                pss = [psb.tile([128, 512], F32, tag="px", name=f"px{g}_{pi}")
                       for pi in range(4)]
                for j in range(4):
                    bi = g * 4 + j
                    t0 = S * bi
                    vsp = wk.tile([128, F], BF16, tag="vsp")
                    nc.sync.dma_start(vsp[:], vpad[t0:t0 + 128, :])
                    for p in range(4):      # head pairs
                        for i in range(2):
                            h = 2 * p + i
                            b_t = bp.tile([W, S], BF16, tag="bt")
                            src = bass.AP(
                                tensor=apad_h,
                                offset=apad_off + (1 + t0) * (H * W) + h * W,
                                ap=[[H * W - 1, S], [1, W]])
                            nc.sync.dma_start_transpose(b_t[:], src)
                            # lhsT = v head-pair [128, 128]; valid out rows are
                            # [i*64:(i+1)*64]; the other half is garbage and
                            # ignored at copyback.
                            nc.tensor.matmul(
                                pss[p][:, j * 128 + i * 64: j * 128 + (i + 1) * 64],
                                vsp[:, p * 128:(p + 1) * 128], b_t[:],
                                start=True, stop=True)
                # copy valid quadrants -> xT (f-major): fold p rows 0:63 = head
                # 2p (cols i=0), rows 64:127 = head 2p+1 (cols i=1)
                for p in range(4):
                    ps3 = pss[p][:].rearrange("d (j i k) -> d j i k", j=4, i=2)
                    dst = xT[:, p, g * 256:(g + 1) * 256] \
                        .rearrange("d (j k) -> d j k", j=4)
                    nc.vector.tensor_copy(out=dst[0:64], in_=ps3[0:64, :, 0, :])
                    nc.vector.tensor_copy(out=dst[64:128], in_=ps3[64:128, :, 1, :])

            # ================= Phase E: out-proj (t-major out) ================
            for tb in range(NT128):
                ps = psm.tile([128, 512], F32, tag="mm")
                for k in range(KF):
                    nc.tensor.matmul(
                        ps[:], xT[:, k, tb * 128:(tb + 1) * 128],
                        wo_t[:, k, :],
                        start=(k == 0), stop=(k == KF - 1))
                o_sb = wk.tile([128, F], F32, tag="osb")
                nc.scalar.copy(o_sb[:], ps[:])
                nc.sync.dma_start(out[tb * 128:(tb + 1) * 128, :], o_sb[:])

    nc.compile()
    return nc


def _get_nc():
    if "nc" not in _CACHE:
        _CACHE["nc"] = _build()
    return _CACHE["nc"]


def kernel(query, key, value, w1, w2, w3, w_out, _trace=False):
    query = np.asarray(query)
    value = np.asarray(value)
    nc = _get_nc()
    bf = ml_dtypes.bfloat16
    w1b = np.ascontiguousarray(np.asarray(w1)).astype(bf)
    w2b = np.ascontiguousarray(np.asarray(w2)).astype(bf)
    w3b = np.ascontiguousarray(np.asarray(w3)).astype(bf)
    wob = np.ascontiguousarray(np.asarray(w_out)).astype(bf)
    in_maps = []
    for b in range(8):
        in_maps.append({
            "qT": np.ascontiguousarray(query[b].T).astype(bf),
            "vT": np.ascontiguousarray(value[b].T).astype(bf),
            "w1": w1b, "w2": w2b, "w3": w3b, "wo": wob,
        })
    res = run_bass_kernel_spmd(nc, in_maps, list(range(8)), trace=_trace)
    if _trace:
        _CACHE["last_result"] = res
    out = np.stack([res.results[b]["out"] for b in range(8)], axis=0)
    return out.astype(np.float32)



# revision 5
# speedup vs baseline: 2.1626x; 2.1626x over previous
"""Trainium2 Bass kernel for LocalDenseSynthesizerAttention.

Data-parallel over batch B=8 -> 8 cores, one batch each. Wire-traffic and
dispatch optimized for the axon tunnel (~90MB/s):
  - jitted executable built once and cached (no per-call retrace)
  - q shipped f-major fp8 (e4m3); v shipped t-major bf16 and transposed
    on-device via XBAR DMA; output returned bf16
  - w1/w2 shipped fp8 scaled x16 (rescaled on device via activation scale),
    w3/w_out bf16; all four sharded 8-way and AllGathered on device
  - donated output buffers created on-device (no zeros upload)

The local window C=45 weighted sum is computed as banded matmuls: the banded
matrix B[s,t'] = attn[t0+t',h,s-t'] is an affine strided view of a zero-padded
attn tensor in DRAM, loaded matmul-ready via XBAR transpose-DMA.

Self-contained: hardcodes shapes from the problem spec.
"""
import sys
sys.path.insert(0, '/opt/trn_rl_repo')
import numpy as np
import ml_dtypes

import concourse.bass as bass
import concourse.mybir as mybir
import concourse.tile as tile
from concourse import bacc
from concourse.bass_utils import run_bass_kernel_spmd

T, F = 2048, 512
H, C, DK = 8, 45, 64
HC = H * C          # 360
W = 128             # padded attn width per head (covers s-t' in [-63,127])
S = 64              # t' band-block size
NB = T // S         # 32 band blocks
PADV = 22           # (C-1)//2
KF = F // 128       # 4 contraction chunks
NT128 = T // 128    # 16
NCORES = 8
FSH = F // NCORES   # 64 weight-shard rows per core

BF16 = mybir.dt.bfloat16
FP8 = mybir.dt.float8e4
F32 = mybir.dt.float32
WSCALE = 16.0       # fp8 weight pre-scale for w1/w2

_CACHE = {}


def _build():
    nc = bacc.Bacc("TRN2", target_bir_lowering=False, debug=False,
                   num_devices=NCORES)
    qT = nc.dram_tensor("qT", (F, T), FP8, kind="ExternalInput")
    v = nc.dram_tensor("v", (T, F), BF16, kind="ExternalInput")
    w1s = nc.dram_tensor("w1s", (FSH, F), FP8, kind="ExternalInput")
    w2s = nc.dram_tensor("w2s", (FSH, HC), FP8, kind="ExternalInput")
    w3s = nc.dram_tensor("w3s", (FSH, F), BF16, kind="ExternalInput")
    wos = nc.dram_tensor("wos", (FSH, F), BF16, kind="ExternalInput")
    out = nc.dram_tensor("out", (T, F), BF16, kind="ExternalOutput")

    # gathered full weights
    w1g = nc.dram_tensor("w1g", (F, F), FP8, kind="Internal")
    w2g = nc.dram_tensor("w2g", (F, HC), FP8, kind="Internal")
    w3g = nc.dram_tensor("w3g", (F, F), BF16, kind="Internal")
    wog = nc.dram_tensor("wog", (F, F), BF16, kind="Internal")

    groups = [list(range(NCORES))]

    with tile.TileContext(nc) as tc:
        with tc.tile_pool(name="wpool", bufs=1) as wp, \
             tc.tile_pool(name="inpool", bufs=1) as inp, \
             tc.tile_pool(name="persist", bufs=1) as pers, \
             tc.tile_pool(name="work", bufs=2) as wk, \
             tc.tile_pool(name="band", bufs=4) as bp, \
             tc.tile_pool(name="psmain", bufs=2, space="PSUM") as psm, \
             tc.tile_pool(name="psband", bufs=4, space="PSUM") as psb, \
             tc.tile_pool(name="drampool", bufs=1, space="DRAM") as dp:

            # ---- AllGather weight shards (rank i fills rows [i*FSH, (i+1)*FSH))
            # collectives cannot read IO tensors: stage shards into Internal
            # DRAM first
            w1si = dp.tile([FSH, F], FP8)
            w2si = dp.tile([FSH, HC], FP8)
            w3si = dp.tile([FSH, F], BF16)
            wosi = dp.tile([FSH, F], BF16)
            for shard, stage, full in ((w1s, w1si, w1g), (w2s, w2si, w2g),
                                       (w3s, w3si, w3g), (wos, wosi, wog)):
                nc.sync.dma_start(stage[:, :], shard[:, :])
                nc.gpsimd.collective_compute(
                    "AllGather", mybir.AluOpType.bypass, groups,
                    [stage[:, :]], [full[:, :]])

            # ---- weights to SBUF, [128, KF, n] layout (partition = contraction)
            w1_t = wp.tile([128, KF, F], FP8, tag="w1")
            nc.sync.dma_start(w1_t[:], w1g[:, :].rearrange("(ko p) n -> p ko n", p=128))
            w2_t = wp.tile([128, KF, HC], FP8, tag="w2")
            nc.sync.dma_start(w2_t[:], w2g[:, :].rearrange("(ko p) n -> p ko n", p=128))
            w3_t = wp.tile([128, KF, F], BF16, tag="w3")
            nc.sync.dma_start(w3_t[:], w3g[:, :].rearrange("(ko p) n -> p ko n", p=128))
            wo_t = wp.tile([128, KF, F], BF16, tag="wo")
            nc.sync.dma_start(wo_t[:], wog[:, :].rearrange("(ko p) n -> p ko n", p=128))

            # ---- q (f-major fp8): plain DMA; v (t-major bf16): XBAR transpose
            qT_t = inp.tile([128, KF, T], FP8, tag="qT")
            nc.sync.dma_start(qT_t[:], qT[:, :].rearrange("(ko p) n -> p ko n", p=128))
            vT_t = inp.tile([128, KF, T], BF16, tag="vT")
            for fo in range(KF):
                eng = nc.scalar if fo % 2 else nc.sync
                eng.dma_start_transpose(vT_t[:, fo, :],
                                        v[:, fo * 128:(fo + 1) * 128])

            # ---- DRAM scratch
            # vpad rows r = t + PADV; rows [0,22) and [2070,2112) zero
            vpad = dp.tile([T + 64, F], BF16)
            # apad: 1 guard row + 2048 data rows + 1 guard row, row = [8 heads x 128]
            apad = dp.tile([T + 2, H * W], BF16)

            # zero tile for guards / edges
            z_t = pers.tile([128, H * W], BF16, tag="zt")
            nc.any.memzero(z_t[:])
            nc.sync.dma_start(vpad[0:PADV, :], z_t[0:PADV, 0:F])
            nc.sync.dma_start(vpad[T + PADV:T + 64, :], z_t[0:64 - PADV, 0:F])
            nc.sync.dma_start(apad[0:1, :], z_t[0:1, :])
            nc.sync.dma_start(apad[T + 1:T + 2, :], z_t[0:1, :])

            # ---- persistent SBUF activations
            qrT = pers.tile([128, KF, T], FP8, tag="qrT")    # relu(q @ w1), f-major
            xT = pers.tile([128, KF, T], BF16, tag="xT")     # band output, f-major

            # ================= Phase A: q-proj + relu (f-major out) ===========
            # PSUM = q @ (16 w1); Relu(psum/16) -> fp8
            for fo in range(KF):
                for tt in range(KF):  # 4 t-tiles of 512
                    ps = psm.tile([128, 512], F32, tag="mm")
                    for k in range(KF):
                        nc.tensor.matmul(
                            ps[:], w1_t[:, k, fo * 128:(fo + 1) * 128],
                            qT_t[:, k, tt * 512:(tt + 1) * 512],
                            start=(k == 0), stop=(k == KF - 1))
                    nc.scalar.activation(qrT[:, fo, tt * 512:(tt + 1) * 512], ps[:],
                                         mybir.ActivationFunctionType.Relu,
                                         scale=1.0 / WSCALE)

            # ================= Phase C: v-proj (t-major out) -> vpad ==========
            for tb in range(NT128):
                ps = psm.tile([128, 512], F32, tag="mm")
                for k in range(KF):
                    nc.tensor.matmul(
                        ps[:], vT_t[:, k, tb * 128:(tb + 1) * 128],
                        w3_t[:, k, :],
                        start=(k == 0), stop=(k == KF - 1))
                v_sb = wk.tile([128, F], BF16, tag="vsb")
                nc.scalar.copy(v_sb[:], ps[:])
                nc.sync.dma_start(vpad[PADV + tb * 128:PADV + (tb + 1) * 128, :], v_sb[:])

            # ====== Phase B: s-proj (t-major) + softmax -> apad (padded) ======
            # PSUM = qr @ (16 w2); Exp(psum/16)
            for tb in range(NT128):
                ps = psm.tile([128, 512], F32, tag="mm")
                for k in range(KF):
                    nc.tensor.matmul(
                        ps[:, 0:HC], qrT[:, k, tb * 128:(tb + 1) * 128],
                        w2_t[:, k, :],
                        start=(k == 0), stop=(k == KF - 1))
                e_t = wk.tile([128, HC], F32, tag="et")
                nc.scalar.activation(e_t[:], ps[:, 0:HC],
                                     mybir.ActivationFunctionType.Exp,
                                     scale=1.0 / WSCALE)
                zs = wk.tile([128, H], F32, tag="zs")
                nc.vector.reduce_sum(zs[:], e_t[:].rearrange("p (h c) -> p h c", c=C),
                                     axis=mybir.AxisListType.X)
                rz = wk.tile([128, H], F32, tag="rz")
                nc.vector.reciprocal(rz[:], zs[:])
                ap_t = wk.tile([128, H * W], BF16, tag="apad")
                if tb < 2:
                    # zero the pad region once per pool slot (bufs=2); the pad
                    # columns are never overwritten afterwards
                    nc.any.memzero(ap_t[:])
                nc.vector.tensor_mul(
                    out=ap_t[:].rearrange("p (h w) -> p h w", w=W)[:, :, 0:C],
                    in0=e_t[:].rearrange("p (h c) -> p h c", c=C),
                    in1=rz[:, :, None].to_broadcast((128, H, C)))
                nc.sync.dma_start(apad[1 + tb * 128:1 + (tb + 1) * 128, :], ap_t[:])

            # ================= Phase D: banded attention matmuls ==============
            # x[t', h*64+d] = sum_s vpad[t0+s, h*64+d] * B_h[s, t']
            # B_h loaded via transpose-DMA of sheared apad view.
            apad_h = apad.tensor  # underlying DRAM handle
            apad_off = apad.offset if isinstance(apad.offset, int) else 0
            for g in range(8):          # groups of 4 band blocks = 256 t'
                pss = [psb.tile([128, 512], F32, tag="px", name=f"px{g}_{pi}")
                       for pi in range(4)]
                for j in range(4):
                    bi = g * 4 + j
                    t0 = S * bi
                    vsp = wk.tile([128, F], BF16, tag="vsp")
                    nc.sync.dma_start(vsp[:], vpad[t0:t0 + 128, :])
                    for p in range(4):      # head pairs
                        for i in range(2):
                            h = 2 * p + i
                            b_t = bp.tile([W, S], BF16, tag="bt")
                            src = bass.AP(
                                tensor=apad_h,
                                offset=apad_off + (1 + t0) * (H * W) + h * W,
                                ap=[[H * W - 1, S], [1, W]])
                            eng = nc.scalar if h % 2 else nc.sync
                            eng.dma_start_transpose(b_t[:], src)
                            # lhsT = v head-pair [128, 128]; valid out rows are
                            # [i*64:(i+1)*64]; the other half is garbage and
                            # ignored at copyback.
                            nc.tensor.matmul(
                                pss[p][:, j * 128 + i * 64: j * 128 + (i + 1) * 64],
                                vsp[:, p * 128:(p + 1) * 128], b_t[:],
                                start=True, stop=True)
                # copy valid quadrants -> xT (f-major): fold p rows 0:63 = head
                # 2p (cols i=0), rows 64:127 = head 2p+1 (cols i=1)
                for p in range(4):
                    ps3 = pss[p][:].rearrange("d (j i k) -> d j i k", j=4, i=2)
                    dst = xT[:, p, g * 256:(g + 1) * 256] \
                        .rearrange("d (j k) -> d j k", j=4)
                    nc.vector.tensor_copy(out=dst[0:64], in_=ps3[0:64, :, 0, :])
                    nc.vector.tensor_copy(out=dst[64:128], in_=ps3[64:128, :, 1, :])

            # ================= Phase E: out-proj (t-major out) ================
            for tb in range(NT128):
                ps = psm.tile([128, 512], F32, tag="mm")
                for k in range(KF):
                    nc.tensor.matmul(
                        ps[:], xT[:, k, tb * 128:(tb + 1) * 128],
                        wo_t[:, k, :],
                        start=(k == 0), stop=(k == KF - 1))
                o_sb = wk.tile([128, F], BF16, tag="osb")
                nc.scalar.copy(o_sb[:], ps[:])
                nc.sync.dma_start(out[tb * 128:(tb + 1) * 128, :], o_sb[:])

    nc.compile()
    return nc


def _get_state():
    if "state" in _CACHE:
        return _CACHE["state"]
    import jax
    import jax.numpy as jnp
    from jax.sharding import Mesh, PartitionSpec, NamedSharding
    from jax.experimental.shard_map import shard_map
    from concourse.bass2jax import (_bass_exec_p, install_neuronx_cc_hook,
                                    partition_id_tensor)

    nc = _build()
    install_neuronx_cc_hook()

    partition_name = (nc.partition_id_tensor.name
                      if nc.partition_id_tensor else None)
    in_names, out_names, out_avals = [], [], []
    for alloc in nc.m.functions[0].allocations:
        if not isinstance(alloc, mybir.MemoryLocationSet):
            continue
        if alloc.kind not in ("ExternalInput", "ExternalOutput"):
            continue
        name = alloc.memorylocations[0].name
        if alloc.kind == "ExternalInput":
            if name != partition_name:
                in_names.append(name)
        else:
            out_avals.append(jax.core.ShapedArray(
                tuple(alloc.tensor_shape), mybir.dt.np(alloc.dtype)))
            out_names.append(name)
    n_params, n_outs = len(in_names), len(out_avals)
    in_names_all = list(in_names) + list(out_names)
    if partition_name is not None:
        in_names_all.append(partition_name)

    def _body(*args):
        operands = list(args)
        if partition_name is not None:
            operands.append(partition_id_tensor())
        return tuple(_bass_exec_p.bind(
            *operands,
            out_avals=tuple(out_avals),
            in_names=tuple(in_names_all),
            out_names=tuple(out_names),
            lowering_input_output_aliases=(),
            sim_require_finite=True,
            sim_require_nnan=True,
            nc=nc))

    devices = jax.devices()[:NCORES]
    mesh = Mesh(np.asarray(devices), ("core",))
    in_specs = (PartitionSpec("core"),) * (n_params + n_outs)
    out_specs = (PartitionSpec("core"),) * n_outs
    donate = tuple(range(n_params, n_params + n_outs))
    sharded = jax.jit(
        shard_map(_body, mesh=mesh, in_specs=in_specs, out_specs=out_specs,
                  check_rep=False),
        donate_argnums=donate, keep_unused=True)
    shard = NamedSharding(mesh, PartitionSpec("core"))
    mkzeros = jax.jit(
        lambda: tuple(jnp.zeros((NCORES * a.shape[0], *a.shape[1:]), a.dtype)
                      for a in out_avals),
        out_shardings=(shard,) * n_outs)

    state = {"nc": nc, "sharded": sharded, "mkzeros": mkzeros,
             "in_names": in_names, "out_avals": out_avals, "shard": shard}
    _CACHE["state"] = state
    return state


def _to_bf16_bits(x32):
    """fp32 -> bf16 via round-half-up on the upper 16 bits (RNE-grade error,
    much faster than ml_dtypes astype). Returns uint16 bit pattern."""
    v = x32.view(np.uint32)
    return np.right_shift(v + np.uint32(0x8000), 16).astype(np.uint16)


def _widen_bf16(bits16, out_shape):
    """bf16 bit pattern -> fp32 exactly by zero-extension."""
    buf = np.zeros(bits16.shape + (2,), np.uint16)
    buf[..., 1] = bits16
    return buf.view(np.float32).reshape(out_shape)


def kernel(query, key, value, w1, w2, w3, w_out, _trace=False):
    import jax
    st = _get_state()
    e4 = ml_dtypes.float8_e4m3
    bf = ml_dtypes.bfloat16
    shard = st["shard"]

    query = np.asarray(query)
    value = np.asarray(value)
    # dispatch device-side zero buffers first (async)
    zeros = st["mkzeros"]()
    # v: cheap bf16 conversion, then async H2D while we convert q
    vb = _to_bf16_bits(value).view(bf).reshape(NCORES * T, F)
    v_dev = jax.device_put(vb, shard)
    # q: quantize then transpose to f-major (8, F, T) -> (8F, T)
    q8 = query.astype(e4)
    qT = np.ascontiguousarray(q8.transpose(0, 2, 1)).reshape(NCORES * F, T)
    q_dev = jax.device_put(qT, shard)
    w1b = (np.asarray(w1) * WSCALE).astype(e4)           # (F, F) global shard
    w2b = (np.asarray(w2) * WSCALE).astype(e4)           # (F, HC)
    w3b = _to_bf16_bits(np.asarray(w3)).view(bf)         # (F, F)
    wob = _to_bf16_bits(np.asarray(w_out)).view(bf)      # (F, F)

    arrays = {"qT": q_dev, "v": v_dev, "w1s": w1b, "w2s": w2b, "w3s": w3b,
              "wos": wob}
    ins = [arrays[name] for name in st["in_names"]]
    outs = st["sharded"](*ins, *zeros)
    out16 = np.asarray(outs[0]).view(np.uint16)
    return _widen_bf16(out16, (NCORES, T, F))


# revision 11
# speedup vs baseline: 2.9346x; 1.3569x over previous
"""Trainium2 Bass kernel for LocalDenseSynthesizerAttention.

Data-parallel over batch B=8 -> 8 cores, one batch each. Wire-traffic and
dispatch optimized for the axon tunnel (~90MB/s each way, full duplex):
  - jitted executables built once and cached (no per-call retrace)
  - q shipped t-major fp8 (e4m3) and transposed on-device (PE transpose);
    v shipped t-major bf16 and transposed on-device via XBAR DMA;
    output returned bf16 and widened exactly on host
  - w1/w2 shipped fp8 scaled x16 (rescaled on device via activation scale),
    w3/w_out bf16; shipped as 8-way shards once per call to a tiny
    weights launch that AllGathers them on device; the full per-core
    weights stay device-resident and feed the compute launches
  - compute is split into sequence chunks (the attention window is local,
    halo = 22), one 8-core launch per chunk: chunk i+1's upload overlaps
    chunk i's exec + download
  - donated output buffers created on-device (no zeros upload)

The local window C=45 weighted sum is computed as banded matmuls: the banded
matrix B[s,t'] = attn[t0+t',h,s-t'] is an affine strided view of a zero-padded
attn tensor in DRAM, loaded matmul-ready via XBAR transpose-DMA.

Self-contained: hardcodes shapes from the problem spec.
"""
import sys
sys.path.insert(0, '/opt/trn_rl_repo')
import numpy as np
import ml_dtypes

import concourse.bass as bass
import concourse.mybir as mybir
import concourse.tile as tile
from concourse import bacc
from concourse import masks

T, F = 2048, 512
H, C, DK = 8, 45, 64
HC = H * C          # 360
W = 128             # padded attn width per head (covers s-t' in [-63,127])
S = 64              # t' band-block size
PADV = 22           # (C-1)//2
KF = F // 128       # 4 contraction chunks
B = 8               # total batches / cores
FSH = F // B        # 64 weight-shard rows per core

NCHUNK = 2
TC = T // NCHUNK    # sequence-chunk length
VH = 64             # v halo rows each side (>= PADV, keeps tiles 128-aligned)
TV = TC + 2 * VH    # v input rows per chunk
VOFF = VH - PADV    # chunk-vpad[r] = v_in[r + VOFF]

BF16 = mybir.dt.bfloat16
FP8 = mybir.dt.float8e4
F32 = mybir.dt.float32
WSCALE = 16.0       # fp8 weight pre-scale for w1/w2

_CACHE = {}


def _build_w():
    """Tiny weights launch: AllGather 8-way weight shards into full
    per-core weights (device-resident outputs)."""
    nc = bacc.Bacc("TRN2", target_bir_lowering=False, debug=False,
                   num_devices=B)
    w1s = nc.dram_tensor("w1s", (FSH, F), FP8, kind="ExternalInput")
    w2s = nc.dram_tensor("w2s", (FSH, HC), FP8, kind="ExternalInput")
    w3s = nc.dram_tensor("w3s", (FSH, F), BF16, kind="ExternalInput")
    wos = nc.dram_tensor("wos", (FSH, F), BF16, kind="ExternalInput")
    w1f = nc.dram_tensor("w1f", (F, F), FP8, kind="ExternalOutput")
    w2f = nc.dram_tensor("w2f", (F, HC), FP8, kind="ExternalOutput")
    w3f = nc.dram_tensor("w3f", (F, F), BF16, kind="ExternalOutput")
    wof = nc.dram_tensor("wof", (F, F), BF16, kind="ExternalOutput")
    groups = [list(range(B))]
    with tile.TileContext(nc) as tc:
        with tc.tile_pool(name="dram", bufs=1, space="DRAM") as dp:
            # collectives cannot read IO tensors: stage shards first
            stages = (dp.tile([FSH, F], FP8, name="st1"),
                      dp.tile([FSH, HC], FP8, name="st2"),
                      dp.tile([FSH, F], BF16, name="st3"),
                      dp.tile([FSH, F], BF16, name="st4"))
            fulls = (dp.tile([F, F], FP8, name="fu1"),
                     dp.tile([F, HC], FP8, name="fu2"),
                     dp.tile([F, F], BF16, name="fu3"),
                     dp.tile([F, F], BF16, name="fu4"))
            for shard, stage, full, out in zip(
                    (w1s, w2s, w3s, wos), stages, fulls,
                    (w1f, w2f, w3f, wof)):
                nc.sync.dma_start(stage[:, :], shard[:, :])
                nc.gpsimd.collective_compute(
                    "AllGather", mybir.AluOpType.bypass, groups,
                    [stage[:, :]], [full[:, :]])
                nc.sync.dma_start(out[:, :], full[:, :])
    nc.compile()
    return nc


def _build_k():
    """Compute launch for one sequence chunk of TC rows."""
    NT128 = TC // 128           # t-tiles in the chunk
    NTV = TV // 128             # t-tiles of the v input (incl halo)
    NB = TC // S                # band blocks
    nc = bacc.Bacc("TRN2", target_bir_lowering=False, debug=False,
                   num_devices=B)
    q = nc.dram_tensor("q", (TC, F), FP8, kind="ExternalInput")
    v = nc.dram_tensor("v", (TV, F), BF16, kind="ExternalInput")
    w1f = nc.dram_tensor("w1f", (F, F), FP8, kind="ExternalInput")
    w2f = nc.dram_tensor("w2f", (F, HC), FP8, kind="ExternalInput")
    w3f = nc.dram_tensor("w3f", (F, F), BF16, kind="ExternalInput")
    wof = nc.dram_tensor("wof", (F, F), BF16, kind="ExternalInput")
    out = nc.dram_tensor("out", (TC, F), BF16, kind="ExternalOutput")

    with tile.TileContext(nc) as tc:
        with tc.tile_pool(name="wpool", bufs=1) as wp, \
             tc.tile_pool(name="inpool", bufs=1) as inp, \
             tc.tile_pool(name="persist", bufs=1) as pers, \
             tc.tile_pool(name="work", bufs=2) as wk, \
             tc.tile_pool(name="band", bufs=4) as bp, \
             tc.tile_pool(name="psmain", bufs=2, space="PSUM") as psm, \
             tc.tile_pool(name="psband", bufs=4, space="PSUM") as psb, \
             tc.tile_pool(name="pstp", bufs=2, space="PSUM") as ptp, \
             tc.tile_pool(name="drampool", bufs=1, space="DRAM") as dp:

            # ---- weights to SBUF, [128, KF, n] layout (partition = contraction)
            w1_t = wp.tile([128, KF, F], FP8, tag="w1")
            nc.sync.dma_start(w1_t[:], w1f[:, :].rearrange("(ko p) n -> p ko n", p=128))
            w2_t = wp.tile([128, KF, HC], FP8, tag="w2")
            nc.sync.dma_start(w2_t[:], w2f[:, :].rearrange("(ko p) n -> p ko n", p=128))
            w3_t = wp.tile([128, KF, F], BF16, tag="w3")
            nc.sync.dma_start(w3_t[:], w3f[:, :].rearrange("(ko p) n -> p ko n", p=128))
            wo_t = wp.tile([128, KF, F], BF16, tag="wo")
            nc.sync.dma_start(wo_t[:], wof[:, :].rearrange("(ko p) n -> p ko n", p=128))

            # ---- v (t-major bf16): XBAR transpose to f-major
            vT_t = inp.tile([128, KF, TV], BF16, tag="vT")
            for fo in range(KF):
                eng = nc.scalar if fo % 2 else nc.sync
                eng.dma_start_transpose(vT_t[:, fo, :],
                                        v[:, fo * 128:(fo + 1) * 128])

            # ---- q (t-major fp8): PE-transpose to f-major
            ident = pers.tile([128, 128], FP8, tag="ident")
            masks.make_identity(nc, ident[:])
            qT_t = inp.tile([128, KF, TC], FP8, tag="qT")
            for tt in range(NT128):
                qstage = wk.tile([128, F], FP8, tag="qstage")
                nc.sync.dma_start(qstage[:], q[tt * 128:(tt + 1) * 128, :])
                for fo in range(KF):
                    # fp8 PE transpose requires output element step of 2
                    pst = ptp.tile([128, 256], FP8, tag="qtp")
                    pstv = pst[:].rearrange("p (a b) -> p a b", b=2)[:, :, 0]
                    nc.tensor.transpose(pstv,
                                        qstage[:, fo * 128:(fo + 1) * 128],
                                        ident[:])
                    nc.scalar.copy(qT_t[:, fo, tt * 128:(tt + 1) * 128],
                                   pstv)

            # ---- DRAM scratch
            # vproj rows j = w3-projection of v_in row j; chunk-vpad[r] = row
            # r + VOFF; v_in's zero halo rows project to exact zeros
            vproj = dp.tile([TV, F], BF16)
            # apad: 1 guard row + TC data rows + 1 guard row, row = [8 heads x 128]
            apad = dp.tile([TC + 2, H * W], BF16)

            # zero tile for apad guards
            z_t = pers.tile([128, H * W], BF16, tag="zt")
            nc.any.memzero(z_t[:])
            nc.sync.dma_start(apad[0:1, :], z_t[0:1, :])
            nc.sync.dma_start(apad[TC + 1:TC + 2, :], z_t[0:1, :])

            # ---- persistent SBUF activations
            qrT = pers.tile([128, KF, TC], FP8, tag="qrT")   # relu(q @ w1), f-major
            xT = pers.tile([128, KF, TC], BF16, tag="xT")    # band output, f-major

            # ================= Phase A: q-proj + relu (f-major out) ===========
            # PSUM = q @ (16 w1); Relu(psum/16) -> fp8
            for fo in range(KF):
                for tt in range(TC // 512):
                    ps = psm.tile([128, 512], F32, tag="mm")
                    for k in range(KF):
                        nc.tensor.matmul(
                            ps[:], w1_t[:, k, fo * 128:(fo + 1) * 128],
                            qT_t[:, k, tt * 512:(tt + 1) * 512],
                            start=(k == 0), stop=(k == KF - 1))
                    nc.scalar.activation(qrT[:, fo, tt * 512:(tt + 1) * 512], ps[:],
                                         mybir.ActivationFunctionType.Relu,
                                         scale=1.0 / WSCALE)

            # ================= Phase C: v-proj (t-major out) -> vproj =========
            for tb in range(NTV):
                ps = psm.tile([128, 512], F32, tag="mm")
                for k in range(KF):
                    nc.tensor.matmul(
                        ps[:], vT_t[:, k, tb * 128:(tb + 1) * 128],
                        w3_t[:, k, :],
                        start=(k == 0), stop=(k == KF - 1))
                v_sb = wk.tile([128, F], BF16, tag="vsb")
                nc.scalar.copy(v_sb[:], ps[:])
                nc.sync.dma_start(vproj[tb * 128:(tb + 1) * 128, :], v_sb[:])

            # ====== Phase B: s-proj (t-major) + softmax -> apad (padded) ======
            # PSUM = qr @ (16 w2); Exp(psum/16)
            for tb in range(NT128):
                ps = psm.tile([128, 512], F32, tag="mm")
                for k in range(KF):
                    nc.tensor.matmul(
                        ps[:, 0:HC], qrT[:, k, tb * 128:(tb + 1) * 128],
                        w2_t[:, k, :],
                        start=(k == 0), stop=(k == KF - 1))
                e_t = wk.tile([128, HC], F32, tag="et")
                nc.scalar.activation(e_t[:], ps[:, 0:HC],
                                     mybir.ActivationFunctionType.Exp,
                                     scale=1.0 / WSCALE)
                zs = wk.tile([128, H], F32, tag="zs")
                nc.vector.reduce_sum(zs[:], e_t[:].rearrange("p (h c) -> p h c", c=C),
                                     axis=mybir.AxisListType.X)
                rz = wk.tile([128, H], F32, tag="rz")
                nc.vector.reciprocal(rz[:], zs[:])
                ap_t = wk.tile([128, H * W], BF16, tag="apad")
                if tb < 2:
                    # zero the pad region once per pool slot (bufs=2); the pad
                    # columns are never overwritten afterwards
                    nc.any.memzero(ap_t[:])
                nc.vector.tensor_mul(
                    out=ap_t[:].rearrange("p (h w) -> p h w", w=W)[:, :, 0:C],
                    in0=e_t[:].rearrange("p (h c) -> p h c", c=C),
                    in1=rz[:, :, None].to_broadcast((128, H, C)))
                nc.sync.dma_start(apad[1 + tb * 128:1 + (tb + 1) * 128, :], ap_t[:])

            # ================= Phase D: banded attention matmuls ==============
            # x[t', h*64+d] = sum_s chunkvpad[t0+s, h*64+d] * B_h[s, t']
            # B_h loaded via transpose-DMA of sheared apad view.
            apad_h = apad.tensor  # underlying DRAM handle
            apad_off = apad.offset if isinstance(apad.offset, int) else 0
            for g in range(NB // 4):    # groups of 4 band blocks = 256 t'
                pss = [psb.tile([128, 512], F32, tag="px", name=f"px{g}_{pi}")
                       for pi in range(4)]
                for j in range(4):
                    bi = g * 4 + j
                    t0 = S * bi
                    vsp = wk.tile([128, F], BF16, tag="vsp")
                    nc.sync.dma_start(vsp[:], vproj[VOFF + t0:VOFF + t0 + 128, :])
                    for p in range(4):      # head pairs
                        for i in range(2):
                            h = 2 * p + i
                            b_t = bp.tile([W, S], BF16, tag="bt")
                            src = bass.AP(
                                tensor=apad_h,
                                offset=apad_off + (1 + t0) * (H * W) + h * W,
                                ap=[[H * W - 1, S], [1, W]])
                            eng = nc.scalar if h % 2 else nc.sync
                            eng.dma_start_transpose(b_t[:], src)
                            # lhsT = v head-pair [128, 128]; valid out rows are
                            # [i*64:(i+1)*64]; the other half is garbage and
                            # ignored at copyback.
                            nc.tensor.matmul(
                                pss[p][:, j * 128 + i * 64: j * 128 + (i + 1) * 64],
                                vsp[:, p * 128:(p + 1) * 128], b_t[:],
                                start=True, stop=True)
                # copy valid quadrants -> xT (f-major): fold p rows 0:63 = head
                # 2p (cols i=0), rows 64:127 = head 2p+1 (cols i=1)
                for p in range(4):
                    ps3 = pss[p][:].rearrange("d (j i k) -> d j i k", j=4, i=2)
                    dst = xT[:, p, g * 256:(g + 1) * 256] \
                        .rearrange("d (j k) -> d j k", j=4)
                    nc.vector.tensor_copy(out=dst[0:64], in_=ps3[0:64, :, 0, :])
                    nc.vector.tensor_copy(out=dst[64:128], in_=ps3[64:128, :, 1, :])

            # ================= Phase E: out-proj (t-major out) ================
            for tb in range(NT128):
                ps = psm.tile([128, 512], F32, tag="mm")
                for k in range(KF):
                    nc.tensor.matmul(
                        ps[:], xT[:, k, tb * 128:(tb + 1) * 128],
                        wo_t[:, k, :],
                        start=(k == 0), stop=(k == KF - 1))
                o_sb = wk.tile([128, F], BF16, tag="osb")
                nc.scalar.copy(o_sb[:], ps[:])
                nc.sync.dma_start(out[tb * 128:(tb + 1) * 128, :], o_sb[:])

    nc.compile()
    return nc


def _make_exec(nc, devices):
    """Cached jitted executable + on-device zeros maker for one bass module."""
    import jax
    import jax.numpy as jnp
    from jax.sharding import Mesh, PartitionSpec, NamedSharding
    from jax.experimental.shard_map import shard_map
    from concourse.bass2jax import _bass_exec_p, partition_id_tensor

    partition_name = (nc.partition_id_tensor.name
                      if nc.partition_id_tensor else None)
    in_names, out_names, out_avals = [], [], []
    for alloc in nc.m.functions[0].allocations:
        if not isinstance(alloc, mybir.MemoryLocationSet):
            continue
        if alloc.kind not in ("ExternalInput", "ExternalOutput"):
            continue
        name = alloc.memorylocations[0].name
        if alloc.kind == "ExternalInput":
            if name != partition_name:
                in_names.append(name)
        else:
            out_avals.append(jax.core.ShapedArray(
                tuple(alloc.tensor_shape), mybir.dt.np(alloc.dtype)))
            out_names.append(name)
    n_params, n_outs = len(in_names), len(out_avals)
    in_names_all = list(in_names) + list(out_names)
    if partition_name is not None:
        in_names_all.append(partition_name)

    def _body(*args):
        operands = list(args)
        if partition_name is not None:
            operands.append(partition_id_tensor())
        return tuple(_bass_exec_p.bind(
            *operands,
            out_avals=tuple(out_avals),
            in_names=tuple(in_names_all),
            out_names=tuple(out_names),
            lowering_input_output_aliases=(),
            sim_require_finite=True,
            sim_require_nnan=True,
            nc=nc))

    n = len(devices)
    mesh = Mesh(np.asarray(devices), ("core",))
    in_specs = (PartitionSpec("core"),) * (n_params + n_outs)
    out_specs = (PartitionSpec("core"),) * n_outs
    donate = tuple(range(n_params, n_params + n_outs))
    sharded = jax.jit(
        shard_map(_body, mesh=mesh, in_specs=in_specs, out_specs=out_specs,
                  check_rep=False),
        donate_argnums=donate, keep_unused=True)
    shard = NamedSharding(mesh, PartitionSpec("core"))
    mkzeros = jax.jit(
        lambda: tuple(jnp.zeros((n * a.shape[0], *a.shape[1:]), a.dtype)
                      for a in out_avals),
        out_shardings=(shard,) * n_outs)
    return {"sharded": sharded, "mkzeros": mkzeros, "in_names": in_names,
            "out_names": out_names, "shard": shard, "n": n}


def _get_state():
    if "state" in _CACHE:
        return _CACHE["state"]
    import jax
    from concourse.bass2jax import install_neuronx_cc_hook
    install_neuronx_cc_hook()
    devices = jax.devices()[:B]
    wexec = _make_exec(_build_w(), devices)
    kexec = _make_exec(_build_k(), devices)
    state = {"w": wexec, "k": kexec}
    _CACHE["state"] = state
    return state


def _to_bf16_bits(x32):
    """fp32 -> bf16 via round-half-up on the upper 16 bits (RNE-grade error,
    much faster than ml_dtypes astype). Returns uint16 bit pattern."""
    v = np.ascontiguousarray(x32).view(np.uint32)
    return np.right_shift(v + np.uint32(0x8000), 16).astype(np.uint16)


def kernel(query, key, value, w1, w2, w3, w_out, _trace=False):
    import jax
    st = _get_state()
    e4 = ml_dtypes.float8_e4m3
    bf = ml_dtypes.bfloat16
    wx, kx = st["w"], st["k"]

    query = np.asarray(query)
    value = np.asarray(value)

    # ---- weights launch first: tiny upload, runs while v/q upload
    wzeros = wx["mkzeros"]()
    warrs = {"w1s": (np.asarray(w1) * WSCALE).astype(e4),
             "w2s": (np.asarray(w2) * WSCALE).astype(e4),
             "w3s": _to_bf16_bits(np.asarray(w3)).view(bf),
             "wos": _to_bf16_bits(np.asarray(w_out)).view(bf)}
    wouts = wx["sharded"](*[warrs[n] for n in wx["in_names"]], *wzeros)
    wfull = dict(zip(wx["out_names"], wouts))

    # ---- per-chunk compute launches, pipelined
    pending = []
    for ci in range(NCHUNK):
        c0 = ci * TC
        kzeros = kx["mkzeros"]()
        # v chunk with halo, zero-padded at sequence edges
        vbuf = np.zeros((B, TV, F), np.uint16)
        lo, hi = max(0, c0 - VH), min(T, c0 + TC + VH)
        off = lo - (c0 - VH)
        vbuf[:, off:off + (hi - lo)] = _to_bf16_bits(value[:, lo:hi])
        v_dev = jax.device_put(vbuf.view(bf).reshape(B * TV, F), kx["shard"])
        # q chunk, t-major fp8 (transposed on device)
        q8 = query[:, c0:c0 + TC, :].astype(e4).reshape(B * TC, F)
        q_dev = jax.device_put(q8, kx["shard"])
        arrays = {"q": q_dev, "v": v_dev, "w1f": wfull["w1f"],
                  "w2f": wfull["w2f"], "w3f": wfull["w3f"],
                  "wof": wfull["wof"]}
        ins = [arrays[n] for n in kx["in_names"]]
        pending.append(kx["sharded"](*ins, *kzeros))

    # ---- collect: widen bf16 -> fp32 exactly (zero-extension)
    buf = np.zeros((B, T, F, 2), np.uint16)
    for ci, outs in enumerate(pending):
        c0 = ci * TC
        o16 = np.asarray(outs[0]).view(np.uint16).reshape(B, TC, F)
        buf[:, c0:c0 + TC, :, 1] = o16
    return buf.view(np.float32)[..., 0]
